# revision 4
# baseline (speedup 1.0000x reference)
"""MultiHeadGAT Trainium2 kernel: 8-core batch-parallel, transposed-layout pipeline.

Math: for scores e = lrelu(s_i[n] + s_j[m]), softmax numerator
  p = exp(lrelu(s_i+s_j)) = e^{0.2 s_i} * max(e^{0.8 s_i} * e^{s_j}, e^{0.2 s_j})
The e^{0.2 s_i} row factor cancels in softmax, so on-device we only compute
  q[m, n] = adjT[m, n] * max(Wbc[m, n] * u[m], v[m])
with Wbc = broadcast(e^{0.8 s_i}) (n-varying), u = e^{s_j}, v = e^{0.2 s_j}
(per-partition scalars), which is one fused tensor_scalar (mult+max) plus one
tensor_tensor (mask) per tile. Attention output and row-sum Z come from one
PE matmul with lhsT = [ones | pad | Wh_head]; normalization 1/Z = exp(-ln(Z)).
"""

import sys

sys.path.insert(0, "/opt/trn_rl_repo")

import numpy as np

B, N, IN_DIM, H, HD = 8, 1024, 128, 8, 16
OUT_DIM = H * HD
EPS = 1e-5
NB = N // 128  # 8 m-blocks

_CACHE = {}


def _build_program():
    import concourse.bacc as bacc
    import concourse.mybir as mybir
    import concourse.tile as tile

    F16 = mybir.dt.float16
    F32 = mybir.dt.float32
    AF = mybir.ActivationFunctionType
    OP = mybir.AluOpType

    nc = bacc.Bacc("TRN2", target_bir_lowering=False, debug=False, num_devices=8)

    # ---- I/O ----
    hT = nc.dram_tensor("hT", [128, N], F16, kind="ExternalInput")
    adjT = nc.dram_tensor("adjT", [128, NB * N], F16, kind="ExternalInput")
    wcat = nc.dram_tensor("wcat", [128, 128], F16, kind="ExternalInput")
    adst = nc.dram_tensor("adst", [128, H], F16, kind="ExternalInput")
    arep = nc.dram_tensor("arep", [128, H * 128], F16, kind="ExternalInput")
    sel = nc.dram_tensor("sel", [16, H * 128], F16, kind="ExternalInput")
    eye8 = nc.dram_tensor("eye8", [1, H * H], F32, kind="ExternalInput")
    e16 = nc.dram_tensor("e16", [H, 128], F16, kind="ExternalInput")
    w1c = nc.dram_tensor("w1c", [128, 2 * OUT_DIM], F16, kind="ExternalInput")
    b1c = nc.dram_tensor("b1c", [128, 2], F32, kind="ExternalInput")
    w2c = nc.dram_tensor("w2c", [128, 2 * OUT_DIM], F16, kind="ExternalInput")
    b2c = nc.dram_tensor("b2c", [128, 1], F32, kind="ExternalInput")
    g1c = nc.dram_tensor("g1c", [128, 1], F32, kind="ExternalInput")
    b1lc = nc.dram_tensor("b1lc", [128, 1], F32, kind="ExternalInput")
    g2c = nc.dram_tensor("g2c", [128, 1], F32, kind="ExternalInput")
    b2lc = nc.dram_tensor("b2lc", [128, 1], F32, kind="ExternalInput")
    outT = nc.dram_tensor("outT", [128, N], F32, kind="ExternalOutput")

    with tile.TileContext(nc) as tc:
        with (
            tc.tile_pool(name="const", bufs=1) as cpool,
            tc.tile_pool(name="big", bufs=1) as big,
            tc.tile_pool(name="work", bufs=2) as work,
            tc.tile_pool(name="mid", bufs=1) as mid,
            tc.tile_pool(name="rows", bufs=1) as rows,
        ):
            # ---- load everything ----
            hT_t = cpool.tile([128, N], F16)
            nc.gpsimd.dma_start(hT_t[:], hT[:])
            adjT_t = cpool.tile([128, NB * N], F16)
            nc.gpsimd.dma_start(adjT_t[:], adjT[:])
            wcat_t = cpool.tile([128, 128], F16)
            nc.gpsimd.dma_start(wcat_t[:], wcat[:])
            adst_t = cpool.tile([128, H], F16)
            nc.gpsimd.dma_start(adst_t[:], adst[:])
            arep_t = cpool.tile([128, H * 128], F16)
            nc.gpsimd.dma_start(arep_t[:], arep[:])
            sel_t = cpool.tile([16, H * 128], F16)
            nc.gpsimd.dma_start(sel_t[:], sel[:])
            eye8_t = cpool.tile([1, H * H], F32)
            nc.gpsimd.dma_start(eye8_t[:], eye8[:])
            e16_t = cpool.tile([H, 128], F16)
            nc.gpsimd.dma_start(e16_t[:], e16[:])
            w1_t = cpool.tile([128, 2 * OUT_DIM], F16)
            nc.gpsimd.dma_start(w1_t[:], w1c[:])
            b1_t = cpool.tile([128, 2], F32)
            nc.gpsimd.dma_start(b1_t[:], b1c[:])
            w2_t = cpool.tile([128, 2 * OUT_DIM], F16)
            nc.gpsimd.dma_start(w2_t[:], w2c[:])
            b2_t = cpool.tile([128, 1], F32)
            nc.gpsimd.dma_start(b2_t[:], b2c[:])
            g1_t = cpool.tile([128, 1], F32)
            nc.gpsimd.dma_start(g1_t[:], g1c[:])
            b1l_t = cpool.tile([128, 1], F32)
            nc.gpsimd.dma_start(b1l_t[:], b1lc[:])
            g2_t = cpool.tile([128, 1], F32)
            nc.gpsimd.dma_start(g2_t[:], g2c[:])
            b2l_t = cpool.tile([128, 1], F32)
            nc.gpsimd.dma_start(b2l_t[:], b2lc[:])

            onescol = cpool.tile([128, 1], F16)
            nc.vector.memset(onescol[:], 1.0)
            zbias = cpool.tile([128, 1], F32)
            nc.vector.memset(zbias[:], 1e-4)
            epsbias = cpool.tile([128, 1], F32)
            nc.vector.memset(epsbias[:], EPS)
            onesrow = cpool.tile([1, 128], F32)
            nc.vector.memset(onesrow[:], 1.0)

            # ---- phase 1: WhT, Wh_nat->aug, s-cols(u,v), Wbc ----
            whT_sb = big.tile([128, N], F16)     # Wh^T: [(h,d), n]
            whnat_sb = big.tile([128, NB * 128], F16)  # Wh natural per m-block
            aug = big.tile([128, NB * 384], F16)  # per (mb,h): [ones|0*31|Wh16]
            u_all = big.tile([128, NB * H], F32)
            v_all = big.tile([128, NB * H], F32)
            wbc = big.tile([128, H * N], F16)    # e^{0.8 s_i} bcast, per head

            with tc.tile_pool(name="ps1", bufs=3, space="PSUM") as ps1:
                whT_ps = ps1.tile([128, N], F32, tag="ps1")
                for ch in range(2):
                    nc.tensor.matmul(
                        whT_ps[:, ch * 512:(ch + 1) * 512], wcat_t[:],
                        hT_t[:, ch * 512:(ch + 1) * 512], start=True, stop=True,
                    )
                nc.scalar.activation(whT_sb[:], whT_ps[:], AF.Copy)

                # aug skeleton: zeros + ones columns
                nc.vector.memset(aug[:], 0.0)
                aug4 = aug[:].rearrange("p (m h c) -> p m h c", m=NB, h=H, c=48)
                nc.vector.memset(aug4[:, :, :, 0:1], 1.0)

                for mb in range(NB):
                    wn_ps = ps1.tile([128, 128], F32, tag="ps1")
                    nc.tensor.matmul(
                        wn_ps[:], hT_t[:, mb * 128:(mb + 1) * 128], wcat_t[:],
                        start=True, stop=True,
                    )
                    nc.scalar.activation(
                        whnat_sb[:, mb * 128:(mb + 1) * 128], wn_ps[:], AF.Copy
                    )
                    sc_ps = ps1.tile([128, H], F32, tag="ps1")
                    nc.tensor.matmul(
                        sc_ps[:], whT_sb[:, mb * 128:(mb + 1) * 128], adst_t[:],
                        start=True, stop=True,
                    )
                    nc.scalar.activation(
                        u_all[:, mb * H:(mb + 1) * H], sc_ps[:], AF.Exp, scale=1.0
                    )
                    nc.scalar.activation(
                        v_all[:, mb * H:(mb + 1) * H], sc_ps[:], AF.Exp, scale=0.2
                    )
                for mb in range(NB):
                    for hh in range(H):
                        nc.vector.tensor_copy(
                            aug[:, mb * 384 + hh * 48 + 32: mb * 384 + hh * 48 + 48],
                            whnat_sb[:, mb * 128 + hh * 16: mb * 128 + hh * 16 + 16],
                        )
                for hh in range(H):
                    wb_ps = ps1.tile([128, N], F32, tag="ps1")
                    for ch in range(2):
                        nc.tensor.matmul(
                            wb_ps[:, ch * 512:(ch + 1) * 512],
                            arep_t[:, hh * 128:(hh + 1) * 128],
                            whT_sb[:, ch * 512:(ch + 1) * 512],
                            start=True, stop=True,
                        )
                    nc.scalar.activation(
                        wbc[:, hh * N:(hh + 1) * N], wb_ps[:], AF.Exp, scale=0.8
                    )

            # ---- phase 2: attention ----
            stage_all = big.tile([16, H * N], F16)
            with (
                tc.tile_pool(name="ps48", bufs=2, space="PSUM") as ps48,
                tc.tile_pool(name="psacc", bufs=1, space="PSUM") as psacc,
                tc.tile_pool(name="psz", bufs=1, space="PSUM") as psz,
            ):
                asm_ps = psacc.tile([128, N], F32)
                zall_ps = psz.tile([H, N], F32)
                for hh in range(H):
                    q_all = work.tile([128, NB * N], F16, tag="q")
                    for mb in range(NB):
                        t1 = work.tile([128, N], F16, tag="t1")
                        nc.vector.tensor_scalar(
                            t1[:], wbc[:, hh * N:(hh + 1) * N],
                            u_all[:, mb * H + hh: mb * H + hh + 1],
                            v_all[:, mb * H + hh: mb * H + hh + 1],
                            op0=OP.mult, op1=OP.max,
                        )
                        nc.vector.tensor_tensor(
                            q_all[:, mb * N:(mb + 1) * N], t1[:],
                            adjT_t[:, mb * N:(mb + 1) * N], op=OP.mult,
                        )
                    at_ps = ps48.tile([48, N], F32)
                    for mb in range(NB):
                        for ch in range(2):
                            nc.tensor.matmul(
                                at_ps[:, ch * 512:(ch + 1) * 512],
                                aug[:, mb * 384 + hh * 48: mb * 384 + hh * 48 + 48],
                                q_all[:, mb * N + ch * 512: mb * N + ch * 512 + 512],
                                start=(mb == 0), stop=(mb == NB - 1),
                            )
                    nc.scalar.activation(
                        stage_all[:, hh * N:(hh + 1) * N], at_ps[32:48, :], AF.Copy
                    )
                    lnz = work.tile([1, N], F32, tag="lnz")
                    nc.scalar.activation(lnz[:], at_ps[0:1, :], AF.Ln, bias=zbias[0:1, :])
                    for ch in range(2):
                        nc.tensor.matmul(
                            zall_ps[:, ch * 512:(ch + 1) * 512],
                            eye8_t[0:1, hh * H:(hh + 1) * H],
                            lnz[0:1, ch * 512:(ch + 1) * 512],
                            start=(hh == 0), stop=(hh == H - 1),
                        )
                        nc.tensor.matmul(
                            asm_ps[:, ch * 512:(ch + 1) * 512],
                            sel_t[:, hh * 128:(hh + 1) * 128],
                            stage_all[:, hh * N + ch * 512: hh * N + ch * 512 + 512],
                            start=(hh == 0), stop=(hh == H - 1),
                        )

                zinv_all = work.tile([H, N], F16, tag="zinv")
                nc.scalar.activation(zinv_all[:], zall_ps[:], AF.Exp, scale=-1.0)
                stage_full = big.tile([128, N], F16)
                nc.scalar.activation(stage_full[:], asm_ps[:], AF.Copy)

            with tc.tile_pool(name="ps3", bufs=2, space="PSUM") as ps3:
                zbc_ps = ps3.tile([128, N], F32, tag="ps3")
                for ch in range(2):
                    nc.tensor.matmul(
                        zbc_ps[:, ch * 512:(ch + 1) * 512], e16_t[:],
                        zinv_all[:, ch * 512:(ch + 1) * 512], start=True, stop=True,
                    )
                zbcf = big.tile([128, N], F16)
                nc.scalar.activation(zbcf[:], zbc_ps[:], AF.Copy)

                hh_t = big.tile([128, N], F16)
                nc.vector.tensor_tensor(hh_t[:], stage_full[:], zbcf[:], op=OP.mult)
                x_res = big.tile([128, N], F16)
                nc.vector.tensor_tensor(x_res[:], hh_t[:], hT_t[:], op=OP.add)

                # ---- LN1 (transposed: stats via PE over partitions) ----
                def layernorm_T(x_in, g_col, b_col, out_tile):
                    x2 = mid.tile([128, N], F16, tag="x2")
                    nc.vector.tensor_tensor(x2[:], x_in[:], x_in[:], op=OP.mult)
                    mu_ps = ps3.tile([1, N], F32, tag="psrow")
                    msq_ps = ps3.tile([1, N], F32, tag="psrow")
                    x16 = x_in
                    for ch in range(2):
                        nc.tensor.matmul(
                            mu_ps[:, ch * 512:(ch + 1) * 512], onescol[:],
                            x16[:, ch * 512:(ch + 1) * 512], start=True, stop=True,
                        )
                        nc.tensor.matmul(
                            msq_ps[:, ch * 512:(ch + 1) * 512], onescol[:],
                            x2[:, ch * 512:(ch + 1) * 512], start=True, stop=True,
                        )
                    mu = rows.tile([1, N], F32, tag="r1")
                    nc.scalar.activation(mu[:], mu_ps[:], AF.Copy, scale=1.0 / 128)
                    msq = rows.tile([1, N], F32, tag="r2")
                    nc.scalar.activation(msq[:], msq_ps[:], AF.Copy, scale=1.0 / 128)
                    mu2 = rows.tile([1, N], F32, tag="r3")
                    nc.scalar.activation(mu2[:], mu[:], AF.Square)
                    var = rows.tile([1, N], F32, tag="r4")
                    nc.vector.tensor_tensor(var[:], msq[:], mu2[:], op=OP.subtract)
                    lnv = rows.tile([1, N], F32, tag="r5")
                    nc.scalar.activation(lnv[:], var[:], AF.Ln, bias=epsbias[0:1, :])
                    rstd = rows.tile([1, N], F32, tag="r6")
                    nc.scalar.activation(rstd[:], lnv[:], AF.Exp, scale=-0.5)
                    brow = rows.tile([1, N], F32, tag="r7")
                    nc.vector.tensor_tensor(brow[:], mu[:], rstd[:], op=OP.mult)
                    a_ps = ps3.tile([128, N], F32, tag="ps3")
                    b_ps = ps3.tile([128, N], F32, tag="ps3")
                    for ch in range(2):
                        nc.tensor.matmul(
                            a_ps[:, ch * 512:(ch + 1) * 512], onesrow[:],
                            rstd[0:1, ch * 512:(ch + 1) * 512], start=True, stop=True,
                        )
                        nc.tensor.matmul(
                            b_ps[:, ch * 512:(ch + 1) * 512], onesrow[:],
                            brow[0:1, ch * 512:(ch + 1) * 512], start=True, stop=True,
                        )
                    a_bc = mid.tile([128, N], F16, tag="abc")
                    nc.scalar.activation(a_bc[:], a_ps[:], AF.Copy)
                    b_bc = mid.tile([128, N], F16, tag="bbc")
                    nc.scalar.activation(b_bc[:], b_ps[:], AF.Copy)
                    t_ = mid.tile([128, N], F16, tag="lnt")
                    nc.vector.tensor_tensor(t_[:], x_in[:], a_bc[:], op=OP.mult)
                    xn = mid.tile([128, N], F16, tag="lnxn")
                    nc.vector.tensor_tensor(xn[:], t_[:], b_bc[:], op=OP.subtract)
                    nc.vector.tensor_scalar(
                        out_tile[:], xn[:], g_col[:], b_col[:], op0=OP.mult, op1=OP.add
                    )

                xc = big.tile([128, N], F16)
                layernorm_T(x_res, g1_t, b1l_t, xc)

                # ---- FFN ----
                y1s = big.tile([128, 2 * N], F16)
                for cb in range(2):
                    y1_ps = ps3.tile([128, N], F32, tag="ps3")
                    for ch in range(2):
                        nc.tensor.matmul(
                            y1_ps[:, ch * 512:(ch + 1) * 512],
                            w1_t[:, cb * 128:(cb + 1) * 128],
                            xc[:, ch * 512:(ch + 1) * 512], start=True, stop=True,
                        )
                    nc.scalar.activation(
                        y1s[:, cb * N:(cb + 1) * N], y1_ps[:], AF.Relu,
                        bias=b1_t[:, cb:cb + 1],
                    )
                y2_ps = ps3.tile([128, N], F32, tag="ps3")
                for cb in range(2):
                    for ch in range(2):
                        nc.tensor.matmul(
                            y2_ps[:, ch * 512:(ch + 1) * 512],
                            w2_t[:, cb * 128:(cb + 1) * 128],
                            y1s[:, cb * N + ch * 512: cb * N + ch * 512 + 512],
                            start=(cb == 0), stop=(cb == 1),
                        )
                y2b = big.tile([128, N], F16)
                nc.scalar.activation(y2b[:], y2_ps[:], AF.Identity, bias=b2_t[:])
                z_res = big.tile([128, N], F16)
                nc.vector.tensor_tensor(z_res[:], y2b[:], xc[:], op=OP.add)

                # ---- LN2 -> output ----
                outT_sb = big.tile([128, N], F32)
                layernorm_T(z_res, g2_t, b2l_t, outT_sb)
                nc.gpsimd.dma_start(outT[:], outT_sb[:])

    nc.compile()
    return nc


def _host_prep(h, adj_mask, W, a, ln1_g, ln1_b, w1, b1, w2, b2, ln2_g, ln2_b):
    f16 = np.float16
    f32 = np.float32
    # shared weights
    wcat = np.ascontiguousarray(np.transpose(np.asarray(W, f32), (1, 0, 2)).reshape(128, 128)).astype(f16)
    a = np.asarray(a, f32)
    a_src, a_dst = a[:, :HD], a[:, HD:]
    adst = np.zeros((128, H), f16)
    arep = np.zeros((128, H * 128), f16)
    for hh in range(H):
        adst[hh * HD:(hh + 1) * HD, hh] = a_dst[hh].astype(f16)
        arep[hh * HD:(hh + 1) * HD, hh * 128:(hh + 1) * 128] = (
            a_src[hh].astype(f16)[:, None]
        )
    sel = np.zeros((16, H * 128), f16)
    for hh in range(H):
        sel[np.arange(16), hh * 128 + hh * 16 + np.arange(16)] = 1.0
    eye8 = np.zeros((1, H * H), f32)
    eye8[0, np.arange(H) * H + np.arange(H)] = 1.0
    e16 = np.zeros((H, 128), f16)
    for hh in range(H):
        e16[hh, hh * 16:(hh + 1) * 16] = 1.0
    w1c = np.asarray(w1, f32).astype(f16)                      # [128, 256]
    b1c = np.asarray(b1, f32).reshape(2, 128).T.copy()          # [128, 2]
    w2c = np.ascontiguousarray(np.asarray(w2, f32).reshape(2, 128, 128).transpose(1, 0, 2).reshape(128, 256)).astype(f16)
    b2c = np.asarray(b2, f32).reshape(128, 1).copy()
    g1c = np.asarray(ln1_g, f32).reshape(128, 1).copy()
    b1lc = np.asarray(ln1_b, f32).reshape(128, 1).copy()
    g2c = np.asarray(ln2_g, f32).reshape(128, 1).copy()
    b2lc = np.asarray(ln2_b, f32).reshape(128, 1).copy()

    shared = dict(wcat=wcat, adst=adst, arep=arep, sel=sel, eye8=eye8, e16=e16,
                  w1c=w1c, b1c=b1c, w2c=w2c, b2c=b2c, g1c=g1c, b1lc=b1lc,
                  g2c=g2c, b2lc=b2lc)

    h = np.asarray(h, f32)
    adj = np.asarray(adj_mask)
    in_maps = []
    for b in range(B):
        hT = np.ascontiguousarray(h[b].T).astype(f16)                  # [128, 1024]
        adjT = np.ascontiguousarray(
            (adj[b] != 0).T.astype(f16).reshape(NB, 128, N).transpose(1, 0, 2).reshape(128, NB * N)
        )
        in_maps.append(dict(hT=hT, adjT=adjT, **shared))
    return in_maps


def kernel(**inputs):
    from concourse.bass_utils import run_bass_kernel_spmd

    if "nc" not in _CACHE:
        _CACHE["nc"] = _build_program()
    nc = _CACHE["nc"]

    in_maps = _host_prep(**inputs)
    res = run_bass_kernel_spmd(nc, in_maps, list(range(B)))
    out = np.empty((B, N, OUT_DIM), np.float32)
    for b in range(B):
        out[b] = res.results[b]["outT"].T
    return out
